# revision 33
# baseline (speedup 1.0000x reference)
"""GCN encoder (2-layer, mu/logstd heads) on 8 Trainium2 NeuronCores.

Strategy (1D graph partitioning, dst-partitioned edges):
  - Host: add self-loops, fold the full symmetric normalization
    norm = deg^-1/2[s] * w * deg^-1/2[d] into per-edge weights (f64), build a
    load-balancing node permutation (round-robin deal by degree into blocks of
    128 lanes spread over 8 cores), sort each block's edges by source row for
    HBM locality, and lay out per-core edge metadata: int16 gather indices
    (wrapped-16 SWDGE layout, lo/hi table split for the int16 range), per-edge
    dst lane + normalized edge weight.
  - Device (single SPMD program, TileContext):
      Phase A: every core projects the full x @ W1 (8-tile slabs; batched
               slab-sized writes of the bf16 node-major gather table).
      Phase B: per window of blocks: dma_gather edge source rows (split
               across SWDGE queues) -> edge-major SBUF tiles; build scaled
               one-hot matrices omega[e,n] = (iota==dst_lane)*norm with one
               dual-op tensor_scalar per tile; PE matmuls accumulate
               sum_e omega[e,n]*msg[e,f] per 128-node block in PSUM
               (= the segment_sum); bias via a K=1 matmul; ReLU flush;
               PE-transpose h1; immediately project with [Wmu||Wls]
               (phase C fused) and stage bf16 rows; every CHUNK blocks fire
               a chunked AllGather of table2 so communication overlaps the
               remaining message passing.
      Phase D: same message passing against the chunk-major table2, fused
               mu||logstd (64+64 columns), f32 outputs staged in SBUF and
               written with two strided DMAs per chunk.
  - Host: inverse-permute rows, return (mu, logstd).
"""

import os
import sys

sys.path.insert(0, "/opt/trn_rl_repo")

import numpy as np
import ml_dtypes
from contextlib import ExitStack

import concourse.bass as bass
import concourse.bacc as bacc
import concourse.mybir as mybir
import concourse.tile as tile
from concourse.bass_utils import run_bass_kernel_spmd

P = 128
NCORES = 8
VLO = 32768          # int16 index range per gather table view
WINDOW_BLOCKS = int(os.environ.get("KERNEL_WB", "4"))
NSWQ = int(os.environ.get("KERNEL_NSWQ", "4"))
LOSPLIT = int(os.environ.get("KERNEL_LOSPLIT", "3"))
AG_CHUNKS = int(os.environ.get("KERNEL_AGCH", "4"))

BF16 = mybir.dt.bfloat16
F32 = mybir.dt.float32
I16 = mybir.dt.int16
NPBF16 = ml_dtypes.bfloat16


def _ceil_div(a, b):
    return -(-a // b)


# ----------------------------------------------------------------------------
# Host preprocessing
# ----------------------------------------------------------------------------

def _build_pass_layout(src_rows, e_core, e_brow, e_lane, e_ew, nblk, ranges,
                       n_table_rows):
    """Lay out one message-passing pass. Edges are classed by the table row
    RANGE their source falls in (`ranges`, contiguous ascending); within a
    window each class gets its own gather op against that range's table view
    (so the gather only depends on that range being written — enabling
    producer/consumer overlap), with int16 view-relative indices. Tiles are
    allocated per (block, class) with a max over cores only (the SPMD program
    is shared), and edges within each (core, block, class) group are sorted
    by source row so descriptor streams walk HBM mostly forward."""
    n_edges = len(src_rows)
    NCLS = len(ranges)
    range_ends = np.array([r1 for (_, r1) in ranges], np.int64)
    range_starts = np.array([r0 for (r0, _) in ranges], np.int64)
    assert all(r1 - r0 <= VLO for (r0, r1) in ranges)
    cls = np.searchsorted(range_ends, src_rows, side="right")
    assert (src_rows >= range_starts[cls]).all()

    gid = (e_core * nblk + e_brow) * NCLS + cls
    order = np.lexsort((src_rows, gid))
    gid_s = gid[order]
    counts = np.bincount(gid_s, minlength=NCORES * nblk * NCLS)
    starts = np.concatenate([[0], np.cumsum(counts)[:-1]])
    rank = np.arange(n_edges) - starts[gid_s]

    # tiles per (block, class): max over cores so all cores share the program
    K_br = _ceil_div(counts.reshape(NCORES, nblk, NCLS).max(axis=0), P)

    windows = []
    b = 0
    while b < nblk:
        wb = min(WINDOW_BLOCKS, nblk - b)
        windows.append((b, wb))
        b += wb

    # window-local tile order: for class r: for block j in window: K_br tiles
    lt_of = np.zeros((nblk, NCLS), np.int64)      # window-local first tile
    gt0_of_w = []                                 # global id of window tile 0
    wtiles = []                                   # tiles per window
    gather_ops = []                               # per window: (r, t0, t1, c0)
    mm_tiles = [[] for _ in range(nblk)]          # per block: local tile ids
    col_base = 0
    gbase = 0
    for w, (b0, wb) in enumerate(windows):
        gt0_of_w.append(gbase)
        ops = []
        lt = 0
        for r in range(NCLS):
            t0 = lt
            for j in range(wb):
                brow = b0 + j
                lt_of[brow, r] = lt
                for k in range(int(K_br[brow, r])):
                    mm_tiles[brow].append(lt + k)
                lt += int(K_br[brow, r])
            if lt > t0:
                ops.append((r, t0, lt, col_base + t0 * (P // 16)))
        gather_ops.append(ops)
        wtiles.append(lt)
        col_base += lt * (P // 16)
        gbase += lt
    TOT_TILES = gbase
    TOTCOLS = col_base
    MAXWT = max(wtiles)

    e_core_s = e_core[order]
    e_brow_s = e_brow[order]
    e_lane_s = e_lane[order]
    e_ew_s = e_ew[order]
    src_s = src_rows[order]
    cls_s = cls[order]

    k_local = rank // P
    p_slot = rank % P
    lt_s = lt_of[e_brow_s, cls_s] + k_local       # window-local tile id
    win_of_brow = np.zeros(nblk, np.int64)
    for w, (b0, wb) in enumerate(windows):
        win_of_brow[b0:b0 + wb] = w
    w_s = win_of_brow[e_brow_s]
    gt = np.array(gt0_of_w)[w_s] + lt_s

    dst_slab = np.full((NCORES, P, TOT_TILES), -1.0, np.float32)
    ew_slab = np.zeros((NCORES, P, TOT_TILES), np.float32)
    dst_slab[e_core_s, p_slot, gt] = e_lane_s.astype(np.float32)
    ew_slab[e_core_s, p_slot, gt] = e_ew_s.astype(np.float32)

    idx = np.zeros((NCORES, 16, TOTCOLS), np.int16)
    flat = lt_s * P + p_slot                      # window-local flat slot
    wcol0 = np.zeros(len(windows), np.int64)
    acc = 0
    for w in range(len(windows)):
        wcol0[w] = acc
        acc += wtiles[w] * (P // 16)
    col = wcol0[w_s] + flat // 16
    row = flat % 16
    idx[e_core_s, row, col] = (src_s - range_starts[cls_s]).astype(np.int16)

    MAXCT = max((t1 - t0) for ops in gather_ops for (_, t0, t1, _) in ops)

    return dict(
        NCLS=NCLS, ranges=ranges, TOT_TILES=TOT_TILES, MAXWT=MAXWT,
        MAXCT=MAXCT, K_br=K_br, lt_of=lt_of,
        windows=windows, wtiles=wtiles, gt0_of_w=gt0_of_w,
        gather_ops=gather_ops, mm_tiles=mm_tiles,
        dst_slab=dst_slab, ew_slab=ew_slab,
        idx=np.tile(idx, (1, 8, 1)),
        n_table_rows=n_table_rows,
    )


def _preprocess(x, edge_index, weight):
    N = x.shape[0]
    s = edge_index[0].astype(np.int64)
    d = edge_index[1].astype(np.int64)
    w = weight.astype(np.float64)
    s = np.concatenate([s, np.arange(N)])
    d = np.concatenate([d, np.arange(N)])
    w = np.concatenate([w, np.ones(N)])

    deg = np.bincount(d, weights=w, minlength=N)
    dis = np.where(deg > 0, deg ** -0.5, 0.0)
    ew = dis[s] * w * dis[d]          # full symmetric norm folded per edge

    NB = NCORES * _ceil_div(_ceil_div(N, NCORES), P)
    nblk = NB // NCORES
    PAD_CORE = nblk * P
    PAD_N = NB * P

    # balance: round-robin deal nodes (sorted by degree desc) into NB blocks
    tot = np.bincount(d, minlength=N)
    order = np.argsort(-tot, kind="stable")
    blk = np.empty(N, np.int64)
    lane = np.empty(N, np.int64)
    blk[order] = np.arange(N) % NB
    lane[order] = np.arange(N) // NB
    assert lane.max() < P
    core_of = blk // nblk
    brow_of = blk % nblk
    permpos = core_of * PAD_CORE + brow_of * P + lane

    # chunk-major table2 layout: AllGather fires per chunk of CH block-rows,
    # each chunk's output is [core, ch*P, HOUT] at base 8*P*c0
    CH = _ceil_div(nblk, AG_CHUNKS)
    c_of = brow_of // CH
    c0_of = c_of * CH
    ch_of = np.minimum(CH, nblk - c0_of)
    table2pos = (NCORES * P * c0_of + core_of * (ch_of * P)
                 + (brow_of - c0_of) * P + lane)

    e_core = core_of[d]
    e_brow = brow_of[d]
    e_lane = lane[d]

    ROWS1 = _ceil_div(N, P) * P
    NT1 = ROWS1 // P
    # pass-1 classes: quarters of table1's rows, aligned to phase A's write
    # slabs, so each class's gathers only wait for that quarter to be written
    NCLS1 = int(os.environ.get("KERNEL_NCLS1", "4"))
    nt_per = _ceil_div(NT1, NCLS1)
    r1s = []
    for c in range(NCLS1):
        r0 = c * nt_per * P
        r1 = min(ROWS1, (c + 1) * nt_per * P)
        if r0 < r1:
            r1s.append((r0, r1))
    # pass-2 classes: the AllGather chunks, so gathers only wait their chunk
    r2s = []
    for c0 in range(0, nblk, CH):
        ch = min(CH, nblk - c0)
        base = NCORES * c0 * P
        r2s.append((base, base + NCORES * ch * P))

    pass1 = _build_pass_layout(s, e_core, e_brow, e_lane, ew, nblk, r1s, ROWS1)
    pass2 = _build_pass_layout(table2pos[s], e_core, e_brow, e_lane, ew, nblk,
                               r2s, PAD_N)

    return dict(
        N=N, NB=NB, nblk=nblk, PAD_CORE=PAD_CORE, PAD_N=PAD_N, ROWS1=ROWS1,
        CH=CH, permpos=permpos, pass1=pass1, pass2=pass2,
    )


# ----------------------------------------------------------------------------
# Device program
# ----------------------------------------------------------------------------

def _emit_op(nc, pools, pl, table_dram, idx_s, dst_s, ew_s, iota_s,
             op, w, qctr, abl):
    """Emit one (window, class) gather op + its omega builds. Returns
    (msg_tile, omega_tile)."""
    msg_pool, omega_pool = pools["msg"], pools["omega"]
    MAXCT = pools["MAXCT"]
    (r, t0, t1, c0) = op
    nt = t1 - t0
    msg = msg_pool.tile([P, MAXCT, P], BF16, tag="msg")
    omega = omega_pool.tile([P, MAXCT * P], BF16, tag="omega")
    if "nogather" not in abl:
        r0, r1 = pl["ranges"][r]
        n_idx = nt * P
        nc.gpsimd.dma_gather(
            out_ap=msg[:, 0:nt, :],
            in_ap=table_dram[r0:r1, :],
            idxs_ap=idx_s[:, c0:c0 + n_idx // 16],
            num_idxs=n_idx,
            num_idxs_reg=n_idx,
            elem_size=P,
            queue_num=qctr % NSWQ,
            single_packet=(n_idx <= 1024),
        )
    gt0 = pl["gt0_of_w"][w] + t0
    for t in range(nt if "noomega" not in abl else 0):
        nc.vector.tensor_scalar(
            out=omega[:, t * P:(t + 1) * P],
            in0=iota_s,
            scalar1=dst_s[:, gt0 + t:gt0 + t + 1],
            scalar2=ew_s[:, gt0 + t:gt0 + t + 1],
            op0=mybir.AluOpType.is_equal,
            op1=mybir.AluOpType.mult,
        )
    return msg, omega


def _emit_pass(nc, pools, pl, table_dram, idx_s,
               dst_s, ew_s, iota_s, ones_s, bias_s, flush_fn):
    """Window-major pass (used for pass 1): all classes of a window are
    gathered into per-op tiles, each block PSUM-accumulates across its
    classes, then flushes."""
    abl = os.environ.get("KERNEL_ABL", "")
    windows = pl["windows"]
    psum_pool = pools["psum"]
    K_br, lt_of = pl["K_br"], pl["lt_of"]

    qctr = 0
    for w, (b0, wb) in enumerate(windows):
        ops = []   # (r, t0, msg, omega)
        for op in pl["gather_ops"][w]:
            msg, omega = _emit_op(nc, pools, pl, table_dram, idx_s,
                                  dst_s, ew_s, iota_s, op, w, qctr, abl)
            ops.append((op[0], op[1], msg, omega))
            qctr += 1
        for j in range(wb):
            brow = b0 + j
            acc = psum_pool.tile([P, P], F32, tag="acc", space="PSUM")
            first = True
            if "nomm" not in abl:
                for (r, t0, msg, omega) in ops:
                    lt = int(lt_of[brow, r]) - t0
                    for k in range(int(K_br[brow, r])):
                        nc.tensor.matmul(
                            out=acc[:],
                            lhsT=omega[:, (lt + k) * P:(lt + k + 1) * P],
                            rhs=msg[:, lt + k, :], start=first, stop=False)
                        first = False
            nc.tensor.matmul(out=acc[:], lhsT=ones_s, rhs=bias_s,
                             start=first, stop=True)
            flush_fn(brow, acc)


def _emit_pass_classmajor(nc, pools, pl, table_dram, idx_s,
                          dst_s, ew_s, iota_s, ones_s, bias_s,
                          accslab, HOUT, chunk_done_fn):
    """Class-major pass (used for pass 2): sweep all windows for class r
    before moving to class r+1, so each sweep only depends on AllGather
    chunk r — the AG chain hides behind the sweeps. Per-block partials
    accumulate into an SBUF f32 slab; outputs flush from it after the
    final class."""
    abl = os.environ.get("KERNEL_ABL", "")
    windows = pl["windows"]
    psum_pool = pools["psum"]
    K_br, lt_of = pl["K_br"], pl["lt_of"]
    NCLS = pl["NCLS"]
    nblk = K_br.shape[0]

    qctr = 0
    for r in range(NCLS):
        last = (r == NCLS - 1)
        for w, (b0, wb) in enumerate(windows):
            op = next((o for o in pl["gather_ops"][w] if o[0] == r), None)
            if op is not None:
                msg, omega = _emit_op(nc, pools, pl, table_dram, idx_s,
                                      dst_s, ew_s, iota_s, op, w, qctr, abl)
                qctr += 1
            for j in range(wb):
                brow = b0 + j
                kb = int(K_br[brow, r]) if op is not None and "nomm" not in abl else 0
                if r > 0 and kb == 0:
                    if last:
                        chunk_done_fn(brow)
                    continue
                acc = psum_pool.tile([P, HOUT], F32, tag="acc", space="PSUM")
                lt = (int(lt_of[brow, r]) - op[1]) if op is not None else 0
                for k in range(kb):
                    nc.tensor.matmul(
                        out=acc[:],
                        lhsT=omega[:, (lt + k) * P:(lt + k + 1) * P],
                        rhs=msg[:, lt + k, :],
                        start=(k == 0), stop=(r > 0 and k == kb - 1))
                sl = accslab[:, brow * HOUT:(brow + 1) * HOUT]
                if r == 0:
                    nc.tensor.matmul(out=acc[:], lhsT=ones_s, rhs=bias_s,
                                     start=(kb == 0), stop=True)
                    nc.scalar.copy(out=sl, in_=acc[:])
                else:
                    nc.vector.scalar_tensor_tensor(
                        out=sl, in0=acc[:], scalar=1.0, in1=sl,
                        op0=mybir.AluOpType.mult, op1=mybir.AluOpType.add)
                if last:
                    chunk_done_fn(brow)


def _build_program(meta, HID, OUT):
    pl1, pl2 = meta["pass1"], meta["pass2"]
    nblk = meta["nblk"]
    CH = meta["CH"]
    ROWS1, PAD_CORE, PAD_N = meta["ROWS1"], meta["PAD_CORE"], meta["PAD_N"]
    NT1 = ROWS1 // P
    HOUT = 2 * OUT
    abl = os.environ.get("KERNEL_ABL", "")

    scratch = int(os.environ.get("KERNEL_SCRATCH", "16384"))
    nc = bacc.Bacc(num_swdge_queues=NSWQ, dynamic_dma_scratch_size=scratch)
    xT_t = nc.declare_dram_parameter("xT", [P, ROWS1], BF16, isOutput=False)
    W1_t = nc.declare_dram_parameter("W1", [P, HID], BF16, isOutput=False)
    Wcat_t = nc.declare_dram_parameter("Wcat", [HID, HOUT], BF16, isOutput=False)
    b1_t = nc.declare_dram_parameter("b1", [1, HID], BF16, isOutput=False)
    bcat_t = nc.declare_dram_parameter("bcat", [1, HOUT], BF16, isOutput=False)
    iota_t = nc.declare_dram_parameter("iota", [P, P], BF16, isOutput=False)

    ix1_t = nc.declare_dram_parameter("ix1", [P, pl1["idx"].shape[2]], I16, isOutput=False)
    ix2_t = nc.declare_dram_parameter("ix2", [P, pl2["idx"].shape[2]], I16, isOutput=False)
    dst1_t = nc.declare_dram_parameter("dst1", [P, pl1["TOT_TILES"]], F32, isOutput=False)
    ew1_t = nc.declare_dram_parameter("ew1", [P, pl1["TOT_TILES"]], F32, isOutput=False)
    dst2_t = nc.declare_dram_parameter("dst2", [P, pl2["TOT_TILES"]], F32, isOutput=False)
    ew2_t = nc.declare_dram_parameter("ew2", [P, pl2["TOT_TILES"]], F32, isOutput=False)

    mu_t = nc.declare_dram_parameter("mu", [PAD_CORE, OUT], F32, isOutput=True)
    ls_t = nc.declare_dram_parameter("ls", [PAD_CORE, OUT], F32, isOutput=True)

    table1 = nc.dram_tensor("table1", [ROWS1, HID], BF16)
    ag_in = nc.dram_tensor("ag_in", [PAD_CORE, HOUT], BF16)
    table2 = nc.dram_tensor("table2", [PAD_N, HOUT], BF16, addr_space="Shared")

    with tile.TileContext(nc) as tc, ExitStack() as ctx:
        const = ctx.enter_context(tc.tile_pool(name="const", bufs=1))
        xt_pool = ctx.enter_context(tc.tile_pool(name="xt", bufs=3))
        stage_pool = ctx.enter_context(tc.tile_pool(name="stage", bufs=3))
        NBUF = int(os.environ.get("KERNEL_NBUF", "6"))
        msg_pool = ctx.enter_context(tc.tile_pool(name="msg", bufs=NBUF))
        omega_pool = ctx.enter_context(tc.tile_pool(name="omega", bufs=NBUF))
        psum_pool = ctx.enter_context(tc.tile_pool(name="psum", bufs=4, space="PSUM"))
        tp_pool = ctx.enter_context(tc.tile_pool(name="tpsum", bufs=2, space="PSUM"))

        def load_const(param, shape, dtype):
            s = const.tile(shape, dtype, tag=param.name)
            nc.sync.dma_start(out=s[:], in_=param[:])
            return s[:]

        W1_s = load_const(W1_t, [P, HID], BF16)
        Wcat_s = load_const(Wcat_t, [HID, HOUT], BF16)
        b1_s = load_const(b1_t, [1, HID], BF16)
        bcat_s = load_const(bcat_t, [1, HOUT], BF16)
        iota_s = load_const(iota_t, [P, P], BF16)
        ix1_s = load_const(ix1_t, [P, pl1["idx"].shape[2]], I16)
        ix2_s = load_const(ix2_t, [P, pl2["idx"].shape[2]], I16)
        dst1_s = load_const(dst1_t, [P, pl1["TOT_TILES"]], F32)
        ew1_s = load_const(ew1_t, [P, pl1["TOT_TILES"]], F32)
        dst2_s = load_const(dst2_t, [P, pl2["TOT_TILES"]], F32)
        ew2_s = load_const(ew2_t, [P, pl2["TOT_TILES"]], F32)

        ones_s = const.tile([1, P], BF16, tag="ones")
        nc.vector.memset(ones_s[:], 1.0)
        identity_s = const.tile([P, P], BF16, tag="identity")
        nc.vector.memset(identity_s[:], 0.0)
        nc.gpsimd.affine_select(
            out=identity_s[:], in_=identity_s[:],
            compare_op=mybir.AluOpType.not_equal, fill=1.0,
            base=0, pattern=[[-1, P]], channel_multiplier=1)



        h1T = const.tile([P, nblk * P], BF16, tag="h1T")

        # ---- Phase A: full x @ W1, batched slab writes of table1 ----
        XSLAB = int(os.environ.get("KERNEL_XSLAB", "8"))
        for nt0 in range(0, NT1, XSLAB):
            nsl = min(XSLAB, NT1 - nt0)
            xsl = xt_pool.tile([P, XSLAB * P], BF16, tag="xsl")
            nc.sync.dma_start(out=xsl[:, 0:nsl * P],
                              in_=xT_t[:, nt0 * P:(nt0 + nsl) * P])
            slab = stage_pool.tile([P, XSLAB * HID], BF16, tag="t1slab")
            for i in range(nsl):
                pr = psum_pool.tile([P, HID], F32, tag="acc", space="PSUM")
                nc.tensor.matmul(out=pr[:], lhsT=xsl[:, i * P:(i + 1) * P],
                                 rhs=W1_s, start=True, stop=True)
                if i % 2 == 0:
                    nc.scalar.copy(out=slab[:, i * HID:(i + 1) * HID], in_=pr[:])
                else:
                    nc.vector.tensor_scalar(
                        out=slab[:, i * HID:(i + 1) * HID], in0=pr[:],
                        scalar1=1.0, scalar2=None, op0=mybir.AluOpType.mult)
            nc.sync.dma_start(
                out=table1[nt0 * P:(nt0 + nsl) * P, :].rearrange(
                    "(i p) f -> p i f", p=P),
                in_=slab[:, 0:nsl * HID].rearrange("p (i f) -> p i f", f=HID))

        # ---- Phase B (+ fused C and chunked AllGather) ----
        # no barrier: each class's gathers depend only on its table1 quarter
        pools = dict(msg=msg_pool, omega=omega_pool, psum=psum_pool, const=const,
                     MAXCT=max(pl1["MAXCT"], pl2["MAXCT"]))

        ag_state = dict(slab=None, c0=0)

        def flush_ag_chunk(c0, c1):
            ch = c1 - c0
            slab = ag_state["slab"]
            nc.sync.dma_start(
                out=ag_in[c0 * P:c1 * P, :].rearrange("(i p) f -> p i f", p=P),
                in_=slab[:, 0:ch * HOUT].rearrange("p (i f) -> p i f", f=HOUT))
            if "noAG" not in abl:
                base8 = NCORES * c0 * P
                nc.gpsimd.collective_compute(
                    "AllGather", mybir.AluOpType.bypass,
                    replica_groups=[list(range(NCORES))],
                    ins=[ag_in[c0 * P:c1 * P, :]],
                    outs=[table2[base8:base8 + NCORES * ch * P, :]])

        def flush1(brow, acc):
            h1tile = stage_pool.tile([P, HID], BF16, tag="h1tile")
            nc.scalar.activation(out=h1tile[:], in_=acc[:],
                                 func=mybir.ActivationFunctionType.Relu)
            tp = tp_pool.tile([P, P], BF16, tag="tp", space="PSUM")
            nc.tensor.transpose(out=tp[:], in_=h1tile[:], identity=identity_s)
            nc.scalar.copy(out=h1T[:, brow * P:(brow + 1) * P], in_=tp[:])
            # fused phase C: project this block and stage bf16 rows
            if brow % CH == 0:
                agslab = stage_pool.tile([P, CH * HOUT], BF16, tag="agslab")
                ag_state["slab"] = agslab
                ag_state["c0"] = brow
            pr = psum_pool.tile([P, HOUT], F32, tag="acc", space="PSUM")
            nc.tensor.matmul(out=pr[:], lhsT=h1T[:, brow * P:(brow + 1) * P],
                             rhs=Wcat_s, start=True, stop=True)
            o = brow - ag_state["c0"]
            nc.vector.tensor_scalar(
                out=ag_state["slab"][:, o * HOUT:(o + 1) * HOUT], in0=pr[:],
                scalar1=1.0, scalar2=None, op0=mybir.AluOpType.mult)
            if brow == nblk - 1 or brow % CH == CH - 1:
                flush_ag_chunk(ag_state["c0"], brow + 1)

        if "noB" not in abl:
            _emit_pass(nc, pools, pl1, table1, ix1_s,
                       dst1_s, ew1_s, iota_s, ones_s[:], b1_s, flush1)
        else:
            for c in range(0, nblk, CH):
                agslab = stage_pool.tile([P, CH * HOUT], BF16, tag="agslab")
                ag_state["slab"] = agslab
                ag_state["c0"] = c
                nc.vector.memset(ag_state["slab"][:], 0.05)
                flush_ag_chunk(c, min(nblk, c + CH))

        # ---- Phase D (class-major: sweeps hide the AllGather chain) ----
        # no barrier: each class's gathers depend only on its AG chunk
        OCH = 7
        accslab = const.tile([P, nblk * HOUT], F32, tag="accslab")
        o_state = dict(c0=0)

        def chunk_done(brow):
            if brow == nblk - 1 or brow % OCH == OCH - 1:
                c0, c1 = o_state["c0"], brow + 1
                sl = accslab[:, c0 * HOUT:c1 * HOUT].rearrange(
                    "p (i f) -> p i f", f=HOUT)
                nc.sync.dma_start(
                    out=mu_t[c0 * P:c1 * P, :].rearrange("(i p) f -> p i f", p=P),
                    in_=sl[:, :, 0:OUT])
                nc.sync.dma_start(
                    out=ls_t[c0 * P:c1 * P, :].rearrange("(i p) f -> p i f", p=P),
                    in_=sl[:, :, OUT:HOUT])
                o_state["c0"] = brow + 1

        if "noD" not in abl:
            _emit_pass_classmajor(nc, pools, pl2, table2, ix2_s,
                                  dst2_s, ew2_s, iota_s, ones_s[:], bcat_s,
                                  accslab[:], HOUT, chunk_done)

    nc.finalize()
    return nc


# ----------------------------------------------------------------------------
# Public entry
# ----------------------------------------------------------------------------

def _prepare(x, edge_index, weight, W1, b1, Wmu, bmu, Wls, bls):
    x = np.asarray(x)
    N, IN_CH = x.shape
    HID = np.asarray(W1).shape[1]
    OUT = np.asarray(Wmu).shape[1]
    meta = _preprocess(x, np.asarray(edge_index), np.asarray(weight))
    pl1, pl2 = meta["pass1"], meta["pass2"]

    nc = _build_program(meta, HID, OUT)

    xT = np.zeros((P, meta["ROWS1"]), np.float32)
    xT[:IN_CH, :N] = np.asarray(x, np.float32).T
    Wcat = np.concatenate([np.asarray(Wmu), np.asarray(Wls)], axis=1)
    bcat = np.concatenate([np.asarray(bmu), np.asarray(bls)])
    iota = np.tile(np.arange(P, dtype=np.float32)[None, :], (P, 1))

    common = {
        "xT": xT.astype(NPBF16),
        "W1": np.asarray(W1, np.float32).astype(NPBF16),
        "Wcat": Wcat.astype(np.float32).astype(NPBF16),
        "b1": np.asarray(b1, np.float32).astype(NPBF16)[None, :],
        "bcat": bcat.astype(np.float32).astype(NPBF16)[None, :],
        "iota": iota.astype(NPBF16),
    }
    in_maps = []
    for c in range(NCORES):
        m = dict(common)
        m["ix1"] = pl1["idx"][c]
        m["ix2"] = pl2["idx"][c]
        m["dst1"] = pl1["dst_slab"][c]
        m["ew1"] = pl1["ew_slab"][c]
        m["dst2"] = pl2["dst_slab"][c]
        m["ew2"] = pl2["ew_slab"][c]
        in_maps.append(m)
    return nc, in_maps, meta


def _postprocess(results, meta):
    mu_cat = np.concatenate([results[c]["mu"] for c in range(NCORES)])
    ls_cat = np.concatenate([results[c]["ls"] for c in range(NCORES)])
    mu = mu_cat[meta["permpos"]].astype(np.float32)
    ls = ls_cat[meta["permpos"]].astype(np.float32)
    return mu, ls


def _run(x, edge_index, weight, W1, b1, Wmu, bmu, Wls, bls, trace=False):
    nc, in_maps, meta = _prepare(x, edge_index, weight, W1, b1, Wmu, bmu, Wls, bls)
    res = run_bass_kernel_spmd(nc, in_maps, list(range(NCORES)), trace=trace)
    return _postprocess(res.results, meta), res


def kernel(x, edge_index, weight, W1, b1, Wmu, bmu, Wls, bls):
    (mu, ls), _ = _run(x, edge_index, weight, W1, b1, Wmu, bmu, Wls, bls)
    return mu, ls
